# revision 10
# baseline (speedup 1.0000x reference)
"""Trainium2 Bass kernel: sparse (sliding-window) attention block.

Full module per reference:
  RMSNorm -> fused QKV (5120x2880) -> YaRN RoPE -> GQA sliding-window(128)
  causal attention with learned sink logit -> out projection (2880x4096).

Sharding: tensor-parallel over heads across 8 cores. Core c owns q-heads
[8c, 8c+8) and kv-head c. RMSNorm is computed (replicated) on every core.
Each core emits a partial [1024, 2880] output in bf16 (its heads' out-proj
contribution); the host sums the 8 partials in f64 and adds out_b.

V4 design (token-major QKV, interleaved small ops):
  - QKV keeps x k-tiles stationary, streams weights -> token-major [tok, 640]
    output; RMSNorm rescale is a per-partition scalar fused with the bias add
    (one scalar_tensor_tensor drain); RoPE pairs live on the free axis
    (strided-AP muls); v feeds AV untransposed.
  - ssq for RMSNorm rides the DMA-chase phase as PE ones-matmuls.
  - Round-based software pipeline: round r emits scores(r), out-proj(r-2),
    the wave-B chain r+4 — and interleaves every small matmul (AV N=65,
    transposes N=128) between those big matmuls so each LDWEIGHTS hides
    under a long stream; kv chains ride the q chains (shared stationary).
  - DMAs consolidated; first xt/wk group is small so the PE starts early;
    per-tile y written in two half-DMAs to overlap the tail.

Per-core DRAM inputs (host-prearranged, partition-major, contiguous):
  xt    [128, 23, 1024] bf16  x.T k-tiles (zero-padded last tile)
  wk    [128, 23, 640] bf16   qkv weights k-major (cols: 512 q | 64 v | 64 k),
                              pre-scaled by norm_scale
  wout  [128, 4, 2880] bf16   out_w.T shard rhs tiles
  cblob [128, 2952] bf16      bias(640) | mask01(256) | esink(8) |
                              cosq,sinq,cosk,sinkt (4x512, token-major,
                              q tables pre-scaled by sm_scale, sin signed)
Output: y [1024, 2880] bf16 partial.
"""

import math
import sys

import numpy as np

try:
    import concourse.bass as bass
except ImportError:  # pragma: no cover
    sys.path.insert(0, "/opt/trn_rl_repo")
    import concourse.bass as bass

import concourse.bacc as bacc
import concourse.tile as tile
from concourse import mybir
from concourse.masks import make_identity
from concourse.bass_utils import run_bass_kernel_spmd

import ml_dtypes

BF16 = ml_dtypes.bfloat16

T = 1024
HIDDEN = 2880
HD = 64
NH = 64
NKV = 8
SW = 128
NCORES = 8
HPC = NH // NCORES          # q heads per core = 8
QKV_DIM = HD * (NH + 2 * NKV)
SM_SCALE = 1.0 / math.sqrt(HD)

P = 128
KT = 23                      # k-tiles over hidden (zero-padded to 23*128)
NQ = HPC * HD                # 512 q columns per core
NC = NQ + 2 * HD             # 640 qkv columns per core (q | v | k)
MT = T // P                  # 8 token tiles
AW = HD + 1                  # per-head AV width (64 v-dims + denominator)
YC = 480                     # out-proj psum chunk width (6 chunks of 480)
CB = NC + 2 * P + HPC + 4 * MT * HD   # const blob cols = 2952

dt = mybir.dt
AF = mybir.ActivationFunctionType
OP = mybir.AluOpType

_CACHE = {}


# ----------------------------------------------------------------------------
# host-side helpers
# ----------------------------------------------------------------------------

def _rope_cos_sin(num_tokens):
    base = 150000.0
    scaling = 32.0
    init_ctx = 4096.0
    ntk_alpha = 1.0
    ntk_beta = 32.0
    d_half = HD / 2
    freq = base ** (np.arange(0, HD, 2, dtype=np.float32) / HD)
    concentration = 0.1 * math.log(scaling) + 1.0
    low = d_half * math.log(init_ctx / (ntk_beta * 2 * math.pi)) / math.log(base)
    high = d_half * math.log(init_ctx / (ntk_alpha * 2 * math.pi)) / math.log(base)
    interpolation = 1.0 / (scaling * freq)
    extrapolation = 1.0 / freq
    ramp = (np.arange(int(d_half), dtype=np.float32) - low) / (high - low)
    m = 1.0 - np.clip(ramp, 0.0, 1.0)
    inv_freq = interpolation * (1.0 - m) + extrapolation * m
    t = np.arange(num_tokens, dtype=np.float32)
    freqs = t[:, None] * inv_freq[None, :]
    cos = (np.cos(freqs) * concentration).astype(np.float32)
    sin = (np.sin(freqs) * concentration).astype(np.float32)
    return cos, sin  # [T, 32]


def _rope_tables():
    cos, sin = _rope_cos_sin(T)  # [1024, 32]
    cos64 = np.concatenate([cos, cos], axis=1)             # [1024, 64]
    sin64 = np.concatenate([-sin, sin], axis=1)            # signed for halves

    def tok_major(a):  # [1024, 64] -> [128, 8*64]
        return a.reshape(MT, P, HD).transpose(1, 0, 2).reshape(P, MT * HD)

    return (
        tok_major(cos64 * SM_SCALE),
        tok_major(sin64 * SM_SCALE),
        tok_major(cos64),
        tok_major(sin64),
    )


def _prep_core_inputs(core, x, norm_scale, qkv_w, qkv_b, out_w, sinks):
    """Build the per-core input map (all numpy, layouts per module docstring)."""
    q_end = NH * HD
    k_end = q_end + NKV * HD

    # rows of qkv_w for this core: 8 q heads + 1 v head + 1 k head = 640 rows
    qrows = np.arange(core * HPC * HD, (core + 1) * HPC * HD)
    krows = np.arange(q_end + core * HD, q_end + (core + 1) * HD)
    vrows = np.arange(k_end + core * HD, k_end + (core + 1) * HD)
    rows = np.concatenate([qrows, vrows, krows])  # [640]: q | v | k

    wshard = (qkv_w[rows, :] * norm_scale[None, :]).astype(np.float32)
    bshard = qkv_b[rows].astype(np.float32)  # [640]

    def part_major(a2d, ncols):  # [2880, ncols] -> [128, 23, ncols] padded
        out = np.zeros((KT * P, ncols), dtype=np.float32)
        out[:HIDDEN] = a2d
        return out.reshape(KT, P, ncols).transpose(1, 0, 2).astype(BF16).copy()

    wk = part_major(wshard.T, NC)                # [128, 23, 640]
    xt = part_major(x.T.astype(np.float32), T)   # [128, 23, 1024]

    # out_w shard: columns for this core's heads -> [512 hd, 2880 H]
    cols = np.arange(core * HPC * HD, (core + 1) * HPC * HD)
    wo = out_w[:, cols].T.astype(np.float32)
    wout = wo.reshape(4, P, HIDDEN).transpose(1, 0, 2).astype(BF16).copy()

    # const blob: bias | mask01 | esink | rope tables
    j = np.arange(P)[:, None]
    i = np.arange(P)[None, :]
    mask_prev = (j > i).astype(np.float32)
    mask_self = (j <= i).astype(np.float32)
    esink = np.exp(sinks[core * HPC:(core + 1) * HPC].astype(np.float64))
    esink = np.broadcast_to(esink.astype(np.float32), (P, HPC))
    cosq, sinq, cosk, sink_t = _rope_tables()
    cblob = np.concatenate([
        np.broadcast_to(bshard, (P, NC)),
        mask_prev, mask_self, esink, cosq, sinq, cosk, sink_t,
    ], axis=1).astype(BF16)
    assert cblob.shape == (P, CB)

    return {"xt": xt, "wk": wk, "wout": wout, "cblob": cblob.copy()}


# ----------------------------------------------------------------------------
# device kernel (Tile)
# ----------------------------------------------------------------------------

def build_nc():
    nc = bacc.Bacc("TRN2", target_bir_lowering=False, debug=False)

    xt_d = nc.dram_tensor("xt", [P, KT, T], dt.bfloat16, kind="ExternalInput").ap()
    wk_d = nc.dram_tensor("wk", [P, KT, NC], dt.bfloat16, kind="ExternalInput").ap()
    wout_d = nc.dram_tensor("wout", [P, 4, HIDDEN], dt.bfloat16,
                            kind="ExternalInput").ap()
    cblob_d = nc.dram_tensor("cblob", [P, CB], dt.bfloat16,
                             kind="ExternalInput").ap()
    y_d = nc.dram_tensor("y", [T, HIDDEN], dt.bfloat16, kind="ExternalOutput").ap()

    # xt/wk DMA chase groups; small first group so the PE starts early
    KGRP = [(0, 2), (2, 8), (8, 14), (14, 19), (19, KT)]

    def bcast_mid(ap2d, n):
        """[P, F] -> [P, n, F] with a 0-step middle dim (free broadcast)."""
        return bass.AP(tensor=ap2d.tensor, offset=ap2d.offset,
                       ap=[ap2d.ap[0], [0, n]] + list(ap2d.ap[1:]))

    with tile.TileContext(nc) as tc:
        with (
            tc.tile_pool(name="const", bufs=1) as const,
            tc.tile_pool(name="res", bufs=1) as res,
            tc.tile_pool(name="xsqp", bufs=3) as xsqp,
            tc.tile_pool(name="qkvp", bufs=3) as qkvp,
            tc.tile_pool(name="ropep", bufs=2) as ropep,
            tc.tile_pool(name="qrotp", bufs=3) as qrotp,
            tc.tile_pool(name="krotp", bufs=3) as krotp,
            tc.tile_pool(name="qrap", bufs=4) as qrap,
            tc.tile_pool(name="ptp", bufs=8) as ptp,
            tc.tile_pool(name="anormp", bufs=3) as anormp,
            tc.tile_pool(name="atp", bufs=8) as atp,
            tc.tile_pool(name="smallp", bufs=4) as smallp,
            tc.tile_pool(name="ysbp", bufs=2) as ysbp,
            tc.tile_pool(name="pmix", bufs=2, space="PSUM") as pmix,
            tc.tile_pool(name="pq", bufs=4, space="PSUM") as pq,
            tc.tile_pool(name="pkvc", bufs=2, space="PSUM") as pkvc,
        ):
            # ---- DMA issue order: first chase groups, then consts, wout ----
            xt_sb = res.tile([P, KT, T], dt.bfloat16, tag="xt", name="xt")
            wk_sb = res.tile([P, KT, NC], dt.bfloat16, tag="wk", name="wk")
            for g0, g1 in KGRP[:3]:
                nc.sync.dma_start(out=wk_sb[:, g0:g1, :], in_=wk_d[:, g0:g1, :])
                nc.sync.dma_start(out=xt_sb[:, g0:g1, :], in_=xt_d[:, g0:g1, :])
            cb = const.tile([P, CB], dt.bfloat16, tag="cb", name="cb")
            nc.sync.dma_start(out=cb, in_=cblob_d)
            for g0, g1 in KGRP[3:]:
                nc.sync.dma_start(out=wk_sb[:, g0:g1, :], in_=wk_d[:, g0:g1, :])
                nc.sync.dma_start(out=xt_sb[:, g0:g1, :], in_=xt_d[:, g0:g1, :])
            wout_sb = res.tile([P, 4, HIDDEN], dt.bfloat16, tag="wout",
                               name="wout")
            nc.sync.dma_start(out=wout_sb, in_=wout_d)

            # const blob views
            bias_sb = cb[:, 0:NC]
            mask_prev = cb[:, NC:NC + P]
            mask_self = cb[:, NC + P:NC + 2 * P]
            esink_sb = cb[:, NC + 2 * P:NC + 2 * P + HPC]
            tb0 = NC + 2 * P + HPC
            tabs = {}
            for idx, nm in enumerate(("cosq", "sinq", "cosk", "sink_t")):
                tabs[nm] = cb[:, tb0 + idx * MT * HD:tb0 + (idx + 1) * MT * HD] \
                    .rearrange("p (t d) -> p t d", t=MT)

            # ---- constants ----
            ones_b = const.tile([P, 1], dt.bfloat16, tag="ones", name="ones")
            nc.vector.memset(ones_b, 1.0)
            ident_b = const.tile([P, P], dt.bfloat16, tag="identb",
                                 name="identb")
            make_identity(nc, ident_b)
            ident_f = const.tile([1, 1], dt.float32, tag="identf", name="identf")
            nc.vector.memset(ident_f, 1.0)
            zbias = const.tile([P, 1], dt.float32, tag="zbias", name="zbias")
            nc.vector.memset(zbias, 0.0)
            eps_t = const.tile([P, 1], dt.float32, tag="eps", name="eps")
            nc.vector.memset(eps_t, 1e-5)

            # ---- wave A: QKV q-chains 0-3 + kv-chains 0-1 + ssq chase ----
            pq_t = {t: pq.tile([P, NQ], dt.float32, tag="pq", name=f"pq{t}")
                    for t in range(4)}
            kv_t = {t: pkvc.tile([P, NQ], dt.float32, tag="kvc", name=f"kv{t}")
                    for t in range(2)}
            pssq = [pmix.tile([1, 512], dt.float32, tag="mix", name=f"ssq{i}")
                    for i in range(2)]
            for ki in range(KT):
                xsq = xsqp.tile([P, T], dt.bfloat16, tag="xsq", name="xsq")
                nc.vector.tensor_tensor(xsq, xt_sb[:, ki, :], xt_sb[:, ki, :],
                                        OP.mult)
                for i in range(2):
                    nc.tensor.matmul(pssq[i], ones_b,
                                     xsq[:, i * 512:(i + 1) * 512],
                                     start=(ki == 0), stop=(ki == KT - 1))
                for t in range(4):
                    nc.tensor.matmul(pq_t[t], xt_sb[:, ki, t * P:(t + 1) * P],
                                     wk_sb[:, ki, 0:NQ],
                                     start=(ki == 0), stop=(ki == KT - 1))
                    if t < 2:
                        nc.tensor.matmul(kv_t[t][:, 0:2 * HD],
                                         xt_sb[:, ki, t * P:(t + 1) * P],
                                         wk_sb[:, ki, NQ:NC],
                                         start=(ki == 0), stop=(ki == KT - 1))

            # ---- ssq -> rsq8 [128 tok, 8 tile] ----
            ssq_sb = res.tile([1, T], dt.float32, tag="ssq", name="ssq")
            nc.vector.tensor_copy(ssq_sb[:, 0:512], pssq[0])
            nc.scalar.copy(ssq_sb[:, 512:1024], pssq[1])
            ssqT = pmix.tile([P, MT], dt.float32, tag="mix", name="ssqT")
            for t in range(MT):
                nc.tensor.transpose(ssqT[:, t:t + 1],
                                    ssq_sb[0:1, t * P:(t + 1) * P],
                                    ident_f)
            lnm = res.tile([P, MT], dt.float32, tag="lnm", name="lnm")
            nc.scalar.activation(lnm, ssqT, AF.Ln, bias=eps_t, scale=1.0 / HIDDEN)
            rsq8 = res.tile([P, MT], dt.float32, tag="rsq8", name="rsq8")
            nc.scalar.activation(rsq8, lnm, AF.Exp, bias=zbias, scale=-0.5)

            # ---- per-token-tile post-QKV state ----
            krope = res.tile([HD, T], dt.bfloat16, tag="krope", name="krope")
            vtok = [res.tile([P, AW], dt.bfloat16, tag=f"vtok{b}",
                             name=f"vtok{b}")
                    for b in range(MT)]
            qra = {}
            rot = {}       # t -> (qrot, krot) awaiting part2 transposes
            att_tiles = {}
            pts_pend = {}

            def drain_part1(t, pq_tile, pkv_ap):
                """psum -> qkv_sb (fused rsq scale + bias); rope on DVE."""
                rsq = rsq8[:, t:t + 1]
                qkv_sb = qkvp.tile([P, NC], dt.bfloat16, tag="qkv", name="qkv")
                nc.vector.scalar_tensor_tensor(qkv_sb[:, 0:NQ], pq_tile, rsq,
                                               bias_sb[:, 0:NQ], OP.mult,
                                               OP.add)
                nc.vector.scalar_tensor_tensor(qkv_sb[:, NQ:NC], pkv_ap, rsq,
                                               bias_sb[:, NQ:NC], OP.mult,
                                               OP.add)
                nc.gpsimd.tensor_copy(vtok[t][:, 0:HD], qkv_sb[:, NQ:NQ + HD])
                nc.gpsimd.memset(vtok[t][:, HD:HD + 1], 1.0)
                q3 = qkv_sb[:, 0:NQ].rearrange("p (h d) -> p h d", h=HPC)
                m1 = ropep.tile([P, HPC, HD], dt.bfloat16, tag="m1", name="m1")
                m2 = ropep.tile([P, HPC, HD], dt.bfloat16, tag="m2", name="m2")
                cq = tabs["cosq"][:, t, :]
                sq = tabs["sinq"][:, t, :]
                nc.vector.tensor_tensor(m1, q3, bcast_mid(cq, HPC), OP.mult)
                nc.vector.tensor_tensor(m2[:, :, 0:32], q3[:, :, 32:64],
                                        bcast_mid(sq[:, 0:32], HPC), OP.mult)
                nc.vector.tensor_tensor(m2[:, :, 32:64], q3[:, :, 0:32],
                                        bcast_mid(sq[:, 32:64], HPC), OP.mult)
                qrot = qrotp.tile([P, HPC, HD], dt.bfloat16, tag="qrot",
                                  name="qrot")
                nc.vector.tensor_tensor(qrot, m1, m2, OP.add)
                k3 = qkv_sb[:, NQ + HD:NC]
                ck = tabs["cosk"][:, t, :]
                sk = tabs["sink_t"][:, t, :]
                km1 = ropep.tile([P, HD], dt.bfloat16, tag="km1", name="km1")
                km2 = ropep.tile([P, HD], dt.bfloat16, tag="km2", name="km2")
                nc.vector.tensor_tensor(km1, k3, ck, OP.mult)
                nc.vector.tensor_tensor(km2[:, 0:32], k3[:, 32:64],
                                        sk[:, 0:32], OP.mult)
                nc.vector.tensor_tensor(km2[:, 32:64], k3[:, 0:32],
                                        sk[:, 32:64], OP.mult)
                krot = krotp.tile([P, HD], dt.bfloat16, tag="krot", name="krot")
                nc.vector.tensor_tensor(krot, km1, km2, OP.add)
                rot[t] = (qrot, krot)

            def part2_thunks(t):
                """Five PE transposes (+psum drains) building qra[t]/krope."""
                qrot, krot = rot.pop(t)
                qra_t = qrap.tile([HD, HPC, P], dt.bfloat16, tag="qra",
                                  name="qra")
                qra[t] = qra_t
                st = {}

                def tq(hp):
                    def f():
                        if "ptq" not in st:
                            st["ptq"] = pmix.tile([P, 4, P], dt.bfloat16,
                                                  tag="mix", name="ptq")
                        qr2 = qrot.rearrange("p h d -> p (h d)")
                        nc.tensor.transpose(st["ptq"][:, hp, :],
                                            qr2[:, hp * P:(hp + 1) * P],
                                            ident_b)
                        if hp == 3:
                            qv = qra_t.rearrange("p (a b) t -> p a b t", b=2)
                            nc.vector.tensor_copy(qv[:, :, 0, :],
                                                  st["ptq"][0:HD, :, :])
                            nc.vector.tensor_copy(qv[:, :, 1, :],
                                                  st["ptq"][HD:P, :, :])
                    return f

                def tk():
                    ptk = pmix.tile([HD, P], dt.bfloat16, tag="mix", name="ptk")
                    nc.tensor.transpose(ptk, krot, ident_b)
                    nc.scalar.copy(krope[:, t * P:(t + 1) * P], ptk)

                return [tq(0), tq(1), tq(2), tq(3), tk]

            # ---- attention pieces ----
            def attn_scores(b):
                """4 big MMs; exp+mask inline; stashes pt tiles."""
                pts = []
                for kt, msk in ((b - 1, mask_prev), (b, mask_self)):
                    if kt < 0:
                        pts.append(None)
                        continue
                    pt_g = []
                    for g in range(2):
                        ps = pq.tile([P, 4, P], dt.float32, tag="pq",
                                     name="score")
                        nc.tensor.matmul(
                            ps, krope[:, kt * P:(kt + 1) * P],
                            qra[b][:, 4 * g:4 * g + 4, :],
                            start=True, stop=True)
                        pt = ptp.tile([P, 4, P], dt.bfloat16, tag="pt",
                                      name="pt")
                        nc.scalar.activation(pt, ps, AF.Exp, bias=zbias)
                        nc.vector.tensor_tensor(pt, pt, bcast_mid(msk, 4),
                                                OP.mult)
                        pt_g.append(pt)
                    pts.append(pt_g)
                pts_pend[b] = pts

            def av_thunks(b):
                """16 small AV MMs + 4 small transposes, as schedulable thunks."""
                ptA, ptB = pts_pend.pop(b)
                rec8 = smallp.tile([P, HPC], dt.float32, tag="rec8",
                                   name="rec8")
                att = []
                att_tiles[b] = att
                st = {}
                thunks = []

                def av(g, j):
                    def f():
                        if g not in st:
                            st[g] = pmix.tile([P, 4, AW], dt.float32,
                                              tag="mix", name="pg")
                        pg = st[g]
                        if b > 0:
                            nc.tensor.matmul(pg[:, j, :], ptA[g][:, j, :],
                                             vtok[b - 1], start=True,
                                             stop=False)
                            nc.tensor.matmul(pg[:, j, :], ptB[g][:, j, :],
                                             vtok[b], start=False, stop=True)
                        else:
                            nc.tensor.matmul(pg[:, j, :], ptB[g][:, j, :],
                                             vtok[b], start=True, stop=True)
                        if j == 3:
                            g0 = 4 * g
                            nc.vector.tensor_tensor(rec8[:, g0:g0 + 4],
                                                    pg[:, :, HD:HD + 1],
                                                    esink_sb[:, g0:g0 + 4],
                                                    OP.add)
                            nc.vector.reciprocal(rec8[:, g0:g0 + 4],
                                                 rec8[:, g0:g0 + 4])
                            an = anormp.tile([P, 4, HD], dt.bfloat16,
                                             tag="anorm", name="anorm")
                            rec3 = bass.AP(
                                tensor=rec8.tensor,
                                offset=rec8[:, g0:g0 + 4].offset,
                                ap=[rec8.ap[0], [1, 4], [0, HD]])
                            nc.vector.tensor_tensor(an, pg[:, :, 0:HD], rec3,
                                                    OP.mult)
                            st[(g, "an")] = an
                    return f

                def tr(g, j):
                    def f():
                        an = st[(g, "an")]
                        a2 = an.rearrange("p a b -> p (a b)")
                        pat = pmix.tile([P, P], dt.bfloat16, tag="mix",
                                        name="pat")
                        nc.tensor.transpose(pat, a2[:, j * P:(j + 1) * P],
                                            ident_b)
                        at = atp.tile([P, P], dt.bfloat16, tag="at", name="at")
                        if (g + j) % 2 == 0:
                            nc.vector.tensor_copy(at, pat)
                        else:
                            nc.scalar.copy(at, pat)
                        att.append(at)
                    return f

                for g in range(2):
                    for j in range(4):
                        thunks.append(av(g, j))
                thunks += [tr(0, 0), tr(0, 1), tr(1, 0), tr(1, 1)]
                return thunks

            def outproj_thunks(b):
                """12 big MM pairs + psum drains + 2 half y DMAs."""
                att = att_tiles.pop(b)
                ysb = ysbp.tile([P, HIDDEN], dt.bfloat16, tag="ysb", name="ysb")
                st = {}
                thunks = []

                def pair(w0, kk):
                    def f():
                        if w0 not in st:
                            st[w0] = [pq.tile([P, YC], dt.float32, tag="pq",
                                              name="py") for _ in range(2)]
                        pys = st[w0]
                        for ci in range(2):
                            o0 = (w0 + ci) * YC
                            nc.tensor.matmul(pys[ci], att[kk],
                                             wout_sb[:, kk, o0:o0 + YC],
                                             start=(kk == 0), stop=(kk == 3))
                        if kk == 3:
                            for ci in range(2):
                                o0 = (w0 + ci) * YC
                                nc.scalar.activation(ysb[:, o0:o0 + YC],
                                                     pys[ci], AF.Copy)
                            if w0 == 2:
                                nc.sync.dma_start(
                                    out=y_d[b * P:(b + 1) * P, 0:4 * YC],
                                    in_=ysb[:, 0:4 * YC])
                            elif w0 == 4:
                                nc.sync.dma_start(
                                    out=y_d[b * P:(b + 1) * P, 4 * YC:HIDDEN],
                                    in_=ysb[:, 4 * YC:HIDDEN])
                    return f

                for w0 in (0, 2, 4):
                    for kk in range(4):
                        thunks.append(pair(w0, kk))
                return thunks

            def chain_thunks(t):
                """Merged q+kv chain for tile t: 23 big thunks (2 MMs each)."""
                pq_tile = pq.tile([P, NQ], dt.float32, tag="pq", name=f"pq{t}")
                kvt = pkvc.tile([P, NQ], dt.float32, tag="kvc", name=f"kv{t}")
                thunks = []

                def step(ki):
                    def f():
                        lhs = xt_sb[:, ki, t * P:(t + 1) * P]
                        nc.tensor.matmul(pq_tile, lhs, wk_sb[:, ki, 0:NQ],
                                         start=(ki == 0), stop=(ki == KT - 1))
                        nc.tensor.matmul(kvt[:, 0:2 * HD], lhs,
                                         wk_sb[:, ki, NQ:NC],
                                         start=(ki == 0), stop=(ki == KT - 1))
                    return f

                for ki in range(KT):
                    thunks.append(step(ki))
                return thunks, pq_tile, kvt

            def kv_chain(t, pool_tile):
                for ki in range(KT):
                    nc.tensor.matmul(pool_tile[:, 0:2 * HD],
                                     xt_sb[:, ki, t * P:(t + 1) * P],
                                     wk_sb[:, ki, NQ:NC],
                                     start=(ki == 0), stop=(ki == KT - 1))

            def interleave(bigs, smalls):
                """Emit big thunks with smalls injected between them."""
                si = 0
                for i, bg in enumerate(bigs):
                    bg()
                    if si < len(smalls) and i % 1 == 0:
                        smalls[si]()
                        si += 1
                while si < len(smalls):
                    smalls[si]()
                    si += 1

            # ---- pre-round transition: ssq done; drains 0-3; kv 2-3 ----
            drain_part1(0, pq_t[0], kv_t[0][:, 0:2 * HD])
            kv2 = pmix.tile([P, NQ], dt.float32, tag="mix", name="kv2")
            kv_chain(2, kv2)
            drain_part1(1, pq_t[1], kv_t[1][:, 0:2 * HD])
            kv3 = pkvc.tile([P, NQ], dt.float32, tag="kvc", name="kv3")
            kv_chain(3, kv3)
            drain_part1(2, pq_t[2], kv2[:, 0:2 * HD])
            drain_part1(3, pq_t[3], kv3[:, 0:2 * HD])
            for t in (0, 1):
                for f in part2_thunks(t):
                    f()

            # ---- rounds: scores(r) | chain(r+4) | outproj(r-2) big streams,
            #      with av(r-1) + drain-transposes as interleaved smalls ----
            chain_state = {}
            for r in range(10):
                bigs = []
                smalls = []
                if r + 2 <= 3 and (r + 2) in rot:
                    smalls += part2_thunks(r + 2)      # drains 2,3 early
                if 0 <= r - 1 <= 7:
                    smalls += av_thunks(r - 1)
                if r + 3 >= 4 and (r + 3) in rot:
                    smalls += part2_thunks(r + 3)      # wave-B drains
                if r <= 7:
                    sb = r

                    def mk_scores(b):
                        return lambda: attn_scores(b)
                    bigs.append(mk_scores(sb))
                if r - 2 >= 0:
                    bigs += outproj_thunks(r - 2)
                if r + 4 <= 7:
                    cth, cpq, ckv = chain_thunks(r + 4)
                    bigs += cth
                    chain_state[r + 4] = (cpq, ckv)
                interleave(bigs, smalls)
                if r + 4 <= 7:
                    cpq, ckv = chain_state.pop(r + 4)
                    drain_part1(r + 4, cpq, ckv[:, 0:2 * HD])

    nc.compile()
    return nc


# ----------------------------------------------------------------------------
# public entry
# ----------------------------------------------------------------------------

LAST_RESULTS = None


def kernel(x, norm_scale, qkv_w, qkv_b, out_w, out_b, sinks):
    global LAST_RESULTS
    x = np.asarray(x, dtype=np.float32)
    norm_scale = np.asarray(norm_scale, dtype=np.float32)
    qkv_w = np.asarray(qkv_w, dtype=np.float32)
    qkv_b = np.asarray(qkv_b, dtype=np.float32)
    out_w = np.asarray(out_w, dtype=np.float32)
    out_b = np.asarray(out_b, dtype=np.float32)
    sinks = np.asarray(sinks, dtype=np.float32)

    if "nc" not in _CACHE:
        _CACHE["nc"] = build_nc()
    nc = _CACHE["nc"]

    in_maps = [
        _prep_core_inputs(c, x, norm_scale, qkv_w, qkv_b, out_w, sinks)
        for c in range(NCORES)
    ]
    import os
    tmpdir = os.environ.get("BASS_TMPDIR") or None
    res = run_bass_kernel_spmd(nc, in_maps, core_ids=list(range(NCORES)),
                               tmpdir=tmpdir)
    LAST_RESULTS = res
    y = np.zeros((T, HIDDEN), dtype=np.float64)
    for c in range(NCORES):
        y += res.results[c]["y"].astype(np.float64)
    y += out_b.astype(np.float64)[None, :]
    return y.astype(np.float32)


# revision 11
# speedup vs baseline: 1.1881x; 1.1881x over previous
"""Trainium2 Bass kernel: sparse (sliding-window) attention block.

Full module per reference:
  RMSNorm -> fused QKV (5120x2880) -> YaRN RoPE -> GQA sliding-window(128)
  causal attention with learned sink logit -> out projection (2880x4096).

Sharding: tensor-parallel over heads across 8 cores. Core c owns q-heads
[8c, 8c+8) and kv-head c. RMSNorm is computed (replicated) on every core.
Each core emits a partial [1024, 2880] output in bf16 (its heads' out-proj
contribution); the host sums the 8 partials in f64 and adds out_b.

V4 design (token-major QKV, interleaved small ops):
  - QKV keeps x k-tiles stationary, streams weights -> token-major [tok, 640]
    output; RMSNorm rescale is a per-partition scalar fused with the bias add
    (one scalar_tensor_tensor drain); RoPE pairs live on the free axis
    (strided-AP muls); v feeds AV untransposed.
  - ssq for RMSNorm rides the DMA-chase phase as PE ones-matmuls.
  - Round-based software pipeline: round r emits scores(r), out-proj(r-2),
    the wave-B chain r+4 — and interleaves every small matmul (AV N=65,
    transposes N=128) between those big matmuls so each LDWEIGHTS hides
    under a long stream; kv chains ride the q chains (shared stationary).
  - DMAs consolidated; first xt/wk group is small so the PE starts early;
    per-tile y written in two half-DMAs to overlap the tail.

Per-core DRAM inputs (host-prearranged, partition-major, contiguous):
  xt    [128, 23, 1024] bf16  x.T k-tiles (zero-padded last tile)
  wk    [128, 23, 640] bf16   qkv weights k-major (cols: 512 q | 64 v | 64 k),
                              pre-scaled by norm_scale
  wout  [128, 4, 2880] bf16   out_w.T shard rhs tiles
  cblob [128, 2952] bf16      bias(640) | mask01(256) | esink(8) |
                              cosq,sinq,cosk,sinkt (4x512, token-major,
                              q tables pre-scaled by sm_scale, sin signed)
Output: y [1024, 2880] bf16 partial.
"""

import math
import sys

import numpy as np

try:
    import concourse.bass as bass
except ImportError:  # pragma: no cover
    sys.path.insert(0, "/opt/trn_rl_repo")
    import concourse.bass as bass

import concourse.bacc as bacc
import concourse.tile as tile
from concourse import mybir
from concourse.masks import make_identity
from concourse.bass_utils import run_bass_kernel_spmd

import ml_dtypes

BF16 = ml_dtypes.bfloat16

T = 1024
HIDDEN = 2880
HD = 64
NH = 64
NKV = 8
SW = 128
NCORES = 8
HPC = NH // NCORES          # q heads per core = 8
QKV_DIM = HD * (NH + 2 * NKV)
SM_SCALE = 1.0 / math.sqrt(HD)

P = 128
KT = 23                      # k-tiles over hidden (zero-padded to 23*128)
NQ = HPC * HD                # 512 q columns per core
NC = NQ + 2 * HD             # 640 qkv columns per core (q | v | k)
MT = T // P                  # 8 token tiles
AW = HD + 1                  # per-head AV width (64 v-dims + denominator)
YC = 480                     # out-proj psum chunk width (6 chunks of 480)
CB = NC + 2 * P + HPC + 4 * MT * HD   # const blob cols = 2952

dt = mybir.dt
AF = mybir.ActivationFunctionType
OP = mybir.AluOpType

_CACHE = {}


# ----------------------------------------------------------------------------
# host-side helpers
# ----------------------------------------------------------------------------

def _rope_cos_sin(num_tokens):
    base = 150000.0
    scaling = 32.0
    init_ctx = 4096.0
    ntk_alpha = 1.0
    ntk_beta = 32.0
    d_half = HD / 2
    freq = base ** (np.arange(0, HD, 2, dtype=np.float32) / HD)
    concentration = 0.1 * math.log(scaling) + 1.0
    low = d_half * math.log(init_ctx / (ntk_beta * 2 * math.pi)) / math.log(base)
    high = d_half * math.log(init_ctx / (ntk_alpha * 2 * math.pi)) / math.log(base)
    interpolation = 1.0 / (scaling * freq)
    extrapolation = 1.0 / freq
    ramp = (np.arange(int(d_half), dtype=np.float32) - low) / (high - low)
    m = 1.0 - np.clip(ramp, 0.0, 1.0)
    inv_freq = interpolation * (1.0 - m) + extrapolation * m
    t = np.arange(num_tokens, dtype=np.float32)
    freqs = t[:, None] * inv_freq[None, :]
    cos = (np.cos(freqs) * concentration).astype(np.float32)
    sin = (np.sin(freqs) * concentration).astype(np.float32)
    return cos, sin  # [T, 32]


def _rope_tables():
    cos, sin = _rope_cos_sin(T)  # [1024, 32]
    cos64 = np.concatenate([cos, cos], axis=1)             # [1024, 64]
    sin64 = np.concatenate([-sin, sin], axis=1)            # signed for halves

    def tok_major(a):  # [1024, 64] -> [128, 8*64]
        return a.reshape(MT, P, HD).transpose(1, 0, 2).reshape(P, MT * HD)

    return (
        tok_major(cos64 * SM_SCALE),
        tok_major(sin64 * SM_SCALE),
        tok_major(cos64),
        tok_major(sin64),
    )


def _prep_core_inputs(core, x, norm_scale, qkv_w, qkv_b, out_w, sinks):
    """Build the per-core input map (all numpy, layouts per module docstring)."""
    q_end = NH * HD
    k_end = q_end + NKV * HD

    # rows of qkv_w for this core: 8 q heads + 1 v head + 1 k head = 640 rows
    qrows = np.arange(core * HPC * HD, (core + 1) * HPC * HD)
    krows = np.arange(q_end + core * HD, q_end + (core + 1) * HD)
    vrows = np.arange(k_end + core * HD, k_end + (core + 1) * HD)
    rows = np.concatenate([qrows, vrows, krows])  # [640]: q | v | k

    wshard = (qkv_w[rows, :] * norm_scale[None, :]).astype(np.float32)
    bshard = qkv_b[rows].astype(np.float32)  # [640]

    def part_major(a2d, ncols):  # [2880, ncols] -> [128, 23, ncols] padded
        out = np.zeros((KT * P, ncols), dtype=np.float32)
        out[:HIDDEN] = a2d
        return out.reshape(KT, P, ncols).transpose(1, 0, 2).astype(BF16).copy()

    wk = part_major(wshard.T, NC)                # [128, 23, 640]
    xt = part_major(x.T.astype(np.float32), T)   # [128, 23, 1024]

    # out_w shard: columns for this core's heads -> [512 hd, 2880 H]
    cols = np.arange(core * HPC * HD, (core + 1) * HPC * HD)
    wo = out_w[:, cols].T.astype(np.float32)
    wout = wo.reshape(4, P, HIDDEN).transpose(1, 0, 2).astype(BF16).copy()

    # const blob: bias | mask01 | esink | rope tables
    j = np.arange(P)[:, None]
    i = np.arange(P)[None, :]
    mask_prev = (j > i).astype(np.float32)
    mask_self = (j <= i).astype(np.float32)
    esink = np.exp(sinks[core * HPC:(core + 1) * HPC].astype(np.float64))
    esink = np.broadcast_to(esink.astype(np.float32), (P, HPC))
    cosq, sinq, cosk, sink_t = _rope_tables()
    cblob = np.concatenate([
        np.broadcast_to(bshard, (P, NC)),
        mask_prev, mask_self, esink, cosq, sinq, cosk, sink_t,
    ], axis=1).astype(BF16)
    assert cblob.shape == (P, CB)

    return {"xt": xt, "wk": wk, "wout": wout, "cblob": cblob.copy()}


# ----------------------------------------------------------------------------
# device kernel (Tile)
# ----------------------------------------------------------------------------

def build_nc():
    nc = bacc.Bacc("TRN2", target_bir_lowering=False, debug=False)

    xt_d = nc.dram_tensor("xt", [P, KT, T], dt.bfloat16, kind="ExternalInput").ap()
    wk_d = nc.dram_tensor("wk", [P, KT, NC], dt.bfloat16, kind="ExternalInput").ap()
    wout_d = nc.dram_tensor("wout", [P, 4, HIDDEN], dt.bfloat16,
                            kind="ExternalInput").ap()
    cblob_d = nc.dram_tensor("cblob", [P, CB], dt.bfloat16,
                             kind="ExternalInput").ap()
    y_d = nc.dram_tensor("y", [T, HIDDEN], dt.bfloat16, kind="ExternalOutput").ap()

    # xt/wk DMA chase groups; small first group so the PE starts early
    KGRP = [(0, 2), (2, 8), (8, 14), (14, 19), (19, KT)]

    def bcast_mid(ap2d, n):
        """[P, F] -> [P, n, F] with a 0-step middle dim (free broadcast)."""
        return bass.AP(tensor=ap2d.tensor, offset=ap2d.offset,
                       ap=[ap2d.ap[0], [0, n]] + list(ap2d.ap[1:]))

    with tile.TileContext(nc) as tc:
        with (
            tc.tile_pool(name="const", bufs=1) as const,
            tc.tile_pool(name="res", bufs=1) as res,
            tc.tile_pool(name="xsqp", bufs=7) as xsqp,
            tc.tile_pool(name="qkvp", bufs=3) as qkvp,
            tc.tile_pool(name="ropep", bufs=2) as ropep,
            tc.tile_pool(name="qrotp", bufs=3) as qrotp,
            tc.tile_pool(name="krotp", bufs=3) as krotp,
            tc.tile_pool(name="qrap", bufs=4) as qrap,
            tc.tile_pool(name="ptp", bufs=8) as ptp,
            tc.tile_pool(name="anormp", bufs=3) as anormp,
            tc.tile_pool(name="atp", bufs=8) as atp,
            tc.tile_pool(name="smallp", bufs=4) as smallp,
            tc.tile_pool(name="ysbp", bufs=2) as ysbp,
            tc.tile_pool(name="pmix", bufs=2, space="PSUM") as pmix,
            tc.tile_pool(name="pq", bufs=4, space="PSUM") as pq,
            tc.tile_pool(name="pkvc", bufs=2, space="PSUM") as pkvc,
        ):
            # ---- DMA issue order: first chase groups, then consts, wout ----
            xt_sb = res.tile([P, KT, T], dt.bfloat16, tag="xt", name="xt")
            wk_sb = res.tile([P, KT, NC], dt.bfloat16, tag="wk", name="wk")
            for g0, g1 in KGRP[:3]:
                nc.sync.dma_start(out=wk_sb[:, g0:g1, :], in_=wk_d[:, g0:g1, :])
                nc.sync.dma_start(out=xt_sb[:, g0:g1, :], in_=xt_d[:, g0:g1, :])
            cb = const.tile([P, CB], dt.bfloat16, tag="cb", name="cb")
            nc.sync.dma_start(out=cb, in_=cblob_d)
            for g0, g1 in KGRP[3:]:
                nc.sync.dma_start(out=wk_sb[:, g0:g1, :], in_=wk_d[:, g0:g1, :])
                nc.sync.dma_start(out=xt_sb[:, g0:g1, :], in_=xt_d[:, g0:g1, :])
            wout_sb = res.tile([P, 4, HIDDEN], dt.bfloat16, tag="wout",
                               name="wout")
            nc.sync.dma_start(out=wout_sb, in_=wout_d)

            # const blob views
            bias_sb = cb[:, 0:NC]
            mask_prev = cb[:, NC:NC + P]
            mask_self = cb[:, NC + P:NC + 2 * P]
            esink_sb = cb[:, NC + 2 * P:NC + 2 * P + HPC]
            tb0 = NC + 2 * P + HPC
            tabs = {}
            for idx, nm in enumerate(("cosq", "sinq", "cosk", "sink_t")):
                tabs[nm] = cb[:, tb0 + idx * MT * HD:tb0 + (idx + 1) * MT * HD] \
                    .rearrange("p (t d) -> p t d", t=MT)

            # ---- constants ----
            ones_b = const.tile([P, 1], dt.bfloat16, tag="ones", name="ones")
            nc.vector.memset(ones_b, 1.0)
            ident_b = const.tile([P, P], dt.bfloat16, tag="identb",
                                 name="identb")
            make_identity(nc, ident_b)
            ident_f = const.tile([1, 1], dt.float32, tag="identf", name="identf")
            nc.vector.memset(ident_f, 1.0)
            zbias = const.tile([P, 1], dt.float32, tag="zbias", name="zbias")
            nc.vector.memset(zbias, 0.0)
            eps_t = const.tile([P, 1], dt.float32, tag="eps", name="eps")
            nc.vector.memset(eps_t, 1e-5)

            # ---- wave A: QKV q-chains 0-3 + kv-chains 0-1 + ssq chase ----
            pq_t = {t: pq.tile([P, NQ], dt.float32, tag="pq", name=f"pq{t}")
                    for t in range(4)}
            kv_t = {t: pkvc.tile([P, NQ], dt.float32, tag="kvc", name=f"kv{t}")
                    for t in range(2)}
            pssq = [pmix.tile([1, 512], dt.float32, tag="mix", name=f"ssq{i}")
                    for i in range(2)]
            xsqs = {}
            for g0, g1 in KGRP:
                # squares for the group first (DVE), then same-bank MM runs
                for ki in range(g0, g1):
                    xsq = xsqp.tile([P, T], dt.bfloat16, tag="xsq", name="xsq")
                    nc.vector.tensor_tensor(xsq, xt_sb[:, ki, :],
                                            xt_sb[:, ki, :], OP.mult)
                    xsqs[ki] = xsq
                for i in range(2):
                    for ki in range(g0, g1):
                        nc.tensor.matmul(pssq[i], ones_b,
                                         xsqs[ki][:, i * 512:(i + 1) * 512],
                                         start=(ki == 0), stop=(ki == KT - 1))
                for ki in range(g0, g1):
                    xsqs.pop(ki, None)
                for t in range(4):
                    for ki in range(g0, g1):
                        nc.tensor.matmul(pq_t[t],
                                         xt_sb[:, ki, t * P:(t + 1) * P],
                                         wk_sb[:, ki, 0:NQ],
                                         start=(ki == 0), stop=(ki == KT - 1))
                for t in range(2):
                    for ki in range(g0, g1):
                        nc.tensor.matmul(kv_t[t][:, 0:2 * HD],
                                         xt_sb[:, ki, t * P:(t + 1) * P],
                                         wk_sb[:, ki, NQ:NC],
                                         start=(ki == 0), stop=(ki == KT - 1))

            # ---- ssq -> rsq8 [128 tok, 8 tile] ----
            ssq_sb = res.tile([1, T], dt.float32, tag="ssq", name="ssq")
            nc.vector.tensor_copy(ssq_sb[:, 0:512], pssq[0])
            nc.scalar.copy(ssq_sb[:, 512:1024], pssq[1])
            ssqT = pmix.tile([P, MT], dt.float32, tag="mix", name="ssqT")
            for t in range(MT):
                nc.tensor.transpose(ssqT[:, t:t + 1],
                                    ssq_sb[0:1, t * P:(t + 1) * P],
                                    ident_f)
            lnm = res.tile([P, MT], dt.float32, tag="lnm", name="lnm")
            nc.scalar.activation(lnm, ssqT, AF.Ln, bias=eps_t, scale=1.0 / HIDDEN)
            rsq8 = res.tile([P, MT], dt.float32, tag="rsq8", name="rsq8")
            nc.scalar.activation(rsq8, lnm, AF.Exp, bias=zbias, scale=-0.5)

            # ---- per-token-tile post-QKV state ----
            krope = res.tile([HD, T], dt.bfloat16, tag="krope", name="krope")
            vtok = [res.tile([P, AW], dt.bfloat16, tag=f"vtok{b}",
                             name=f"vtok{b}")
                    for b in range(MT)]
            qra = {}
            rot = {}       # t -> (qrot, krot) awaiting part2 transposes
            att_tiles = {}
            pts_pend = {}

            def drain_part1(t, pq_tile, pkv_ap):
                """psum -> qkv_sb (fused rsq scale + bias); rope on DVE."""
                rsq = rsq8[:, t:t + 1]
                qkv_sb = qkvp.tile([P, NC], dt.bfloat16, tag="qkv", name="qkv")
                nc.vector.scalar_tensor_tensor(qkv_sb[:, 0:NQ], pq_tile, rsq,
                                               bias_sb[:, 0:NQ], OP.mult,
                                               OP.add)
                nc.vector.scalar_tensor_tensor(qkv_sb[:, NQ:NC], pkv_ap, rsq,
                                               bias_sb[:, NQ:NC], OP.mult,
                                               OP.add)
                nc.gpsimd.tensor_copy(vtok[t][:, 0:HD], qkv_sb[:, NQ:NQ + HD])
                nc.gpsimd.memset(vtok[t][:, HD:HD + 1], 1.0)
                q3 = qkv_sb[:, 0:NQ].rearrange("p (h d) -> p h d", h=HPC)
                m1 = ropep.tile([P, HPC, HD], dt.bfloat16, tag="m1", name="m1")
                m2 = ropep.tile([P, HPC, HD], dt.bfloat16, tag="m2", name="m2")
                cq = tabs["cosq"][:, t, :]
                sq = tabs["sinq"][:, t, :]
                nc.vector.tensor_tensor(m1, q3, bcast_mid(cq, HPC), OP.mult)
                nc.vector.tensor_tensor(m2[:, :, 0:32], q3[:, :, 32:64],
                                        bcast_mid(sq[:, 0:32], HPC), OP.mult)
                nc.vector.tensor_tensor(m2[:, :, 32:64], q3[:, :, 0:32],
                                        bcast_mid(sq[:, 32:64], HPC), OP.mult)
                qrot = qrotp.tile([P, HPC, HD], dt.bfloat16, tag="qrot",
                                  name="qrot")
                nc.vector.tensor_tensor(qrot, m1, m2, OP.add)
                k3 = qkv_sb[:, NQ + HD:NC]
                ck = tabs["cosk"][:, t, :]
                sk = tabs["sink_t"][:, t, :]
                km1 = ropep.tile([P, HD], dt.bfloat16, tag="km1", name="km1")
                km2 = ropep.tile([P, HD], dt.bfloat16, tag="km2", name="km2")
                nc.vector.tensor_tensor(km1, k3, ck, OP.mult)
                nc.vector.tensor_tensor(km2[:, 0:32], k3[:, 32:64],
                                        sk[:, 0:32], OP.mult)
                nc.vector.tensor_tensor(km2[:, 32:64], k3[:, 0:32],
                                        sk[:, 32:64], OP.mult)
                krot = krotp.tile([P, HD], dt.bfloat16, tag="krot", name="krot")
                nc.vector.tensor_tensor(krot, km1, km2, OP.add)
                rot[t] = (qrot, krot)

            def part2_thunks(t):
                """Five PE transposes (+psum drains) building qra[t]/krope."""
                qrot, krot = rot.pop(t)
                qra_t = qrap.tile([HD, HPC, P], dt.bfloat16, tag="qra",
                                  name="qra")
                qra[t] = qra_t
                st = {}

                def tq(hp):
                    def f():
                        if "ptq" not in st:
                            st["ptq"] = pmix.tile([P, 4, P], dt.bfloat16,
                                                  tag="mix", name="ptq")
                        qr2 = qrot.rearrange("p h d -> p (h d)")
                        nc.tensor.transpose(st["ptq"][:, hp, :],
                                            qr2[:, hp * P:(hp + 1) * P],
                                            ident_b)
                        if hp == 3:
                            qv = qra_t.rearrange("p (a b) t -> p a b t", b=2)
                            nc.vector.tensor_copy(qv[:, :, 0, :],
                                                  st["ptq"][0:HD, :, :])
                            nc.vector.tensor_copy(qv[:, :, 1, :],
                                                  st["ptq"][HD:P, :, :])
                    return f

                def tk():
                    ptk = pmix.tile([HD, P], dt.bfloat16, tag="mix", name="ptk")
                    nc.tensor.transpose(ptk, krot, ident_b)
                    nc.scalar.copy(krope[:, t * P:(t + 1) * P], ptk)

                return [tq(0), tq(1), tq(2), tq(3), tk]

            # ---- attention pieces ----
            def attn_scores(b):
                """4 big MMs; exp+mask inline; stashes pt tiles."""
                pts = []
                for kt, msk in ((b - 1, mask_prev), (b, mask_self)):
                    if kt < 0:
                        pts.append(None)
                        continue
                    pt_g = []
                    for g in range(2):
                        ps = pq.tile([P, 4, P], dt.float32, tag="pq",
                                     name="score")
                        nc.tensor.matmul(
                            ps, krope[:, kt * P:(kt + 1) * P],
                            qra[b][:, 4 * g:4 * g + 4, :],
                            start=True, stop=True)
                        pt = ptp.tile([P, 4, P], dt.bfloat16, tag="pt",
                                      name="pt")
                        nc.scalar.activation(pt, ps, AF.Exp, bias=zbias)
                        nc.vector.tensor_tensor(pt, pt, bcast_mid(msk, 4),
                                                OP.mult)
                        pt_g.append(pt)
                    pts.append(pt_g)
                pts_pend[b] = pts

            def av_thunks(b):
                """16 small AV MMs + 4 small transposes, as schedulable thunks."""
                ptA, ptB = pts_pend.pop(b)
                rec8 = smallp.tile([P, HPC], dt.float32, tag="rec8",
                                   name="rec8")
                att = []
                att_tiles[b] = att
                st = {}
                thunks = []

                def av(g, j):
                    def f():
                        if g not in st:
                            st[g] = pmix.tile([P, 4, AW], dt.float32,
                                              tag="mix", name="pg")
                        pg = st[g]
                        if b > 0:
                            nc.tensor.matmul(pg[:, j, :], ptA[g][:, j, :],
                                             vtok[b - 1], start=True,
                                             stop=False)
                            nc.tensor.matmul(pg[:, j, :], ptB[g][:, j, :],
                                             vtok[b], start=False, stop=True)
                        else:
                            nc.tensor.matmul(pg[:, j, :], ptB[g][:, j, :],
                                             vtok[b], start=True, stop=True)
                        if j == 3:
                            g0 = 4 * g
                            nc.vector.tensor_tensor(rec8[:, g0:g0 + 4],
                                                    pg[:, :, HD:HD + 1],
                                                    esink_sb[:, g0:g0 + 4],
                                                    OP.add)
                            nc.vector.reciprocal(rec8[:, g0:g0 + 4],
                                                 rec8[:, g0:g0 + 4])
                            an = anormp.tile([P, 4, HD], dt.bfloat16,
                                             tag="anorm", name="anorm")
                            rec3 = bass.AP(
                                tensor=rec8.tensor,
                                offset=rec8[:, g0:g0 + 4].offset,
                                ap=[rec8.ap[0], [1, 4], [0, HD]])
                            nc.vector.tensor_tensor(an, pg[:, :, 0:HD], rec3,
                                                    OP.mult)
                            st[(g, "an")] = an
                    return f

                def tr(g, j):
                    def f():
                        an = st[(g, "an")]
                        a2 = an.rearrange("p a b -> p (a b)")
                        pat = pmix.tile([P, P], dt.bfloat16, tag="mix",
                                        name="pat")
                        nc.tensor.transpose(pat, a2[:, j * P:(j + 1) * P],
                                            ident_b)
                        at = atp.tile([P, P], dt.bfloat16, tag="at", name="at")
                        if (g + j) % 2 == 0:
                            nc.vector.tensor_copy(at, pat)
                        else:
                            nc.scalar.copy(at, pat)
                        att.append(at)
                    return f

                for g in range(2):
                    for j in range(4):
                        thunks.append(av(g, j))
                thunks += [tr(0, 0), tr(0, 1), tr(1, 0), tr(1, 1)]
                return thunks

            def outproj_third(b, w0, ysb):
                """Two psum chunks, each a same-bank run of 4 kk MMs."""
                att = att_tiles[b]
                for ci in range(2):
                    o0 = (w0 + ci) * YC
                    py = pq.tile([P, YC], dt.float32, tag="pq", name="py")
                    for kk in range(4):
                        nc.tensor.matmul(py, att[kk],
                                         wout_sb[:, kk, o0:o0 + YC],
                                         start=(kk == 0), stop=(kk == 3))
                    nc.scalar.activation(ysb[:, o0:o0 + YC], py, AF.Copy)
                if w0 == 2:
                    nc.sync.dma_start(out=y_d[b * P:(b + 1) * P, 0:4 * YC],
                                      in_=ysb[:, 0:4 * YC])
                elif w0 == 4:
                    nc.sync.dma_start(out=y_d[b * P:(b + 1) * P,
                                              4 * YC:HIDDEN],
                                      in_=ysb[:, 4 * YC:HIDDEN])
                    att_tiles.pop(b)

            def q_chain(t):
                pq_tile = pq.tile([P, NQ], dt.float32, tag="pq", name=f"pq{t}")
                for ki in range(KT):
                    nc.tensor.matmul(pq_tile, xt_sb[:, ki, t * P:(t + 1) * P],
                                     wk_sb[:, ki, 0:NQ],
                                     start=(ki == 0), stop=(ki == KT - 1))
                return pq_tile

            def kv_chain(t, pool_tile):
                for ki in range(KT):
                    nc.tensor.matmul(pool_tile[:, 0:2 * HD],
                                     xt_sb[:, ki, t * P:(t + 1) * P],
                                     wk_sb[:, ki, NQ:NC],
                                     start=(ki == 0), stop=(ki == KT - 1))

            # ---- pre-round transition: ssq done; drains 0-3; kv 2-3 ----
            drain_part1(0, pq_t[0], kv_t[0][:, 0:2 * HD])
            kv2 = pmix.tile([P, NQ], dt.float32, tag="mix", name="kv2")
            kv_chain(2, kv2)
            drain_part1(1, pq_t[1], kv_t[1][:, 0:2 * HD])
            kv3 = pkvc.tile([P, NQ], dt.float32, tag="kvc", name="kv3")
            kv_chain(3, kv3)
            drain_part1(2, pq_t[2], kv2[:, 0:2 * HD])
            drain_part1(3, pq_t[3], kv3[:, 0:2 * HD])
            for t in (0, 1):
                for f in part2_thunks(t):
                    f()

            # ---- rounds: scores(r); av(r-1); outproj(r-2) in thirds with
            #      pat/part2 transposes between; chain(r+4) q+kv runs ----
            for r in range(10):
                if r <= 7:
                    attn_scores(r)
                av16 = pat4 = None
                if 0 <= r - 1 <= 7:
                    th = av_thunks(r - 1)
                    av16, pat4 = th[:16], th[16:]
                    for f in av16:
                        f()
                ob = r - 2
                ysb = None
                if ob >= 0:
                    ysb = ysbp.tile([P, HIDDEN], dt.bfloat16, tag="ysb",
                                    name="ysb")
                    outproj_third(ob, 0, ysb)
                if pat4 is not None:
                    for f in pat4:
                        f()
                if ob >= 0:
                    outproj_third(ob, 2, ysb)
                if r + 2 <= 3 and (r + 2) in rot:
                    for f in part2_thunks(r + 2):
                        f()
                if r + 3 >= 4 and (r + 3) in rot:
                    for f in part2_thunks(r + 3):
                        f()
                if ob >= 0:
                    outproj_third(ob, 4, ysb)
                if r + 4 <= 7:
                    cpq = q_chain(r + 4)
                    ckv = pkvc.tile([P, NQ], dt.float32, tag="kvc",
                                    name=f"kv{r + 4}")
                    kv_chain(r + 4, ckv)
                    drain_part1(r + 4, cpq, ckv[:, 0:2 * HD])

    nc.compile()
    return nc


# ----------------------------------------------------------------------------
# public entry
# ----------------------------------------------------------------------------

LAST_RESULTS = None


def kernel(x, norm_scale, qkv_w, qkv_b, out_w, out_b, sinks):
    global LAST_RESULTS
    x = np.asarray(x, dtype=np.float32)
    norm_scale = np.asarray(norm_scale, dtype=np.float32)
    qkv_w = np.asarray(qkv_w, dtype=np.float32)
    qkv_b = np.asarray(qkv_b, dtype=np.float32)
    out_w = np.asarray(out_w, dtype=np.float32)
    out_b = np.asarray(out_b, dtype=np.float32)
    sinks = np.asarray(sinks, dtype=np.float32)

    if "nc" not in _CACHE:
        _CACHE["nc"] = build_nc()
    nc = _CACHE["nc"]

    in_maps = [
        _prep_core_inputs(c, x, norm_scale, qkv_w, qkv_b, out_w, sinks)
        for c in range(NCORES)
    ]
    import os
    tmpdir = os.environ.get("BASS_TMPDIR") or None
    res = run_bass_kernel_spmd(nc, in_maps, core_ids=list(range(NCORES)),
                               tmpdir=tmpdir)
    LAST_RESULTS = res
    y = np.zeros((T, HIDDEN), dtype=np.float64)
    for c in range(NCORES):
        y += res.results[c]["y"].astype(np.float64)
    y += out_b.astype(np.float64)[None, :]
    return y.astype(np.float32)
